# revision 25
# baseline (speedup 1.0000x reference)
"""Trainium2 Bass kernel for nn_MixtureOfMambaModel.

Exact graph-level optimization: the classifier head reads x[:, 0] (the cls
token), and every sequence-mixing op in the model is causal (depthwise conv
with left-only padding, forward SSM scan) or per-token (norms, MoE, router).
Token 0 therefore never observes tokens 1..97, and its initial value is
cls_token + modality_embed[:,3] + pos_embed[:,0] — independent of the video /
audio / question inputs. The model output is a function of the weights only,
identical across the batch. The kernel computes that single-token forward
pass exactly, on device, and broadcasts the result to all 16 batch rows.

Device strategy (8 NeuronCores, tensor-parallel single-token forward):
  - All big projections are split 8 ways: in_proj / expert-w1 by output
    columns, out_proj / expert-w2 by contraction rows. The [1024] activation
    vector is replicated as a [128, 8] tile on every core.
  - Three 4KB AllReduces per layer stitch the partials together:
    dt/B/C projections [192], mixer output [1024], weighted MoE output
    [1024]. Collectives run on internal DRAM tiles (CCE fp32 add).
  - Small/serial pieces (rmsnorm, SSM step at t=0, layernorm over 64,
    router top-2, conv tap) are replicated on every core — they are a few
    hundred elements each.
  - Matmuls run stationary-weight with a 1-column moving operand (the
    token), bf16 in / fp32 PSUM accumulation. Biases and norm weights are
    folded host-side exactly as in the dense formulation.
"""

import numpy as np
import ml_dtypes

# ---- model dims (hardcoded per spec) ----
B = 16
D = 1024
INNER = 2048
NS = 64
HID = 4096
E = 4
L = 4
NCLS = 13
DC = D // 128                # 8 chunks of the model dim
NCORES = 8
CIN = 2 * INNER // NCORES // 128   # in-proj col chunks per core (4)
CXM = INNER // NCORES // 128       # xm col chunks per core (2)
CH = HID // NCORES // 128          # expert hidden chunks per core (4)

BF16 = ml_dtypes.bfloat16

_CACHE = {}


# --------------------------------------------------------------------------
# Host-side preparation: slicing / layout / constant folding on weights.
# --------------------------------------------------------------------------

def _prep(inputs):
    f32 = np.float32
    g = {k: np.asarray(v) for k, v in inputs.items()}

    # token-0 initial value: cls + modality_embed[3] + pos_embed[0]
    x0 = (np.asarray(g["cls_token"][0, 0], f32)
          + np.asarray(g["modality_embed"][0, 3], f32)
          + np.asarray(g["pos_embed"][0, 0], f32))            # [1024]

    sh = {}
    sh["x0"] = np.ascontiguousarray(x0.reshape(DC, 128).T).astype(f32)  # [128, 8]

    w_in = (g["in_w"] * g["norm1_w"][:, :, None]).astype(f32)  # [L,1024,4096]
    w_gate = (g["gate_w"] * g["norm2_w"][:, :, None]).astype(f32)
    w_e1 = (g["e_w1"] * g["norm2_w"][:, None, :, None]).astype(f32)
    w_hd = (g["head_w"] * g["fnorm_w"][:, None]).astype(f32)   # [1024, 13]

    # replicated (shared) tensors
    sh["w_gate"] = np.ascontiguousarray(
        w_gate.reshape(L, DC, 128, E).transpose(0, 2, 1, 3)).astype(BF16)
    sh["b_gate"] = g["gate_b"].reshape(L, 1, E).astype(f32)
    sh["b_dtbc"] = np.ascontiguousarray(
        np.stack([g["dt_b"], g["Bp_b"], g["Cp_b"]], axis=2)).astype(f32)
    sh["b_out"] = np.ascontiguousarray(
        g["out_b"].reshape(L, DC, 128).transpose(0, 2, 1)).astype(f32)
    sh["b_e2"] = np.ascontiguousarray(
        g["e_b2"].reshape(L, E, DC, 128).transpose(0, 3, 1, 2)).astype(f32)
    sh["w_hd"] = np.ascontiguousarray(
        w_hd.reshape(DC, 128, NCLS).transpose(1, 0, 2)).astype(BF16)
    sh["b_hd"] = g["head_b"].reshape(1, NCLS).astype(f32)

    percore = []
    for c in range(NCORES):
        pc = {}
        mcols = slice(c * 256, (c + 1) * 256)                  # xm cols
        gcols = slice(INNER + c * 256, INNER + (c + 1) * 256)  # gate cols
        hcols = slice(c * 512, (c + 1) * 512)                  # hidden cols

        wi = np.concatenate([w_in[:, :, mcols], w_in[:, :, gcols]], axis=2)
        # [L, 1024, 512] -> [L, 128p, 8k, 4j, 128m]
        pc["w_in"] = np.ascontiguousarray(
            wi.reshape(L, DC, 128, CIN, 128).transpose(0, 2, 1, 3, 4)
        ).astype(BF16)
        bi = np.concatenate([g["in_b"][:, mcols], g["in_b"][:, gcols]], 1)
        pc["b_in"] = np.ascontiguousarray(
            bi.reshape(L, CIN, 128).transpose(0, 2, 1)).astype(f32)

        cpk = np.zeros((L, 128, CXM, 3), f32)
        cpk[:, :, :, 0] = g["conv_w"][:, mcols, 0, 2].reshape(
            L, CXM, 128).transpose(0, 2, 1)
        cpk[:, :, :, 1] = g["conv_b"][:, mcols].reshape(
            L, CXM, 128).transpose(0, 2, 1)
        cpk[:, :, :, 2] = g["D_param"][:, mcols].reshape(
            L, CXM, 128).transpose(0, 2, 1)
        pc["cpk"] = cpk

        wd = np.concatenate([g["dt_w"], g["Bp_w"], g["Cp_w"]], 2)[:, mcols]
        pc["w_dtbc"] = np.ascontiguousarray(
            wd.reshape(L, CXM, 128, 3 * NS).transpose(0, 2, 1, 3)
        ).astype(BF16)                                         # [L,128,2,192]

        s2 = np.concatenate(
            [g["s2i_w"][:, :, mcols], g["s2i_b"][:, None, mcols]], 1)
        pc["w_s2i"] = np.ascontiguousarray(s2).astype(BF16)    # [L, 65, 256]

        pc["w_out"] = np.ascontiguousarray(
            g["out_w"][:, mcols].reshape(L, CXM, 128, DC, 128)
            .transpose(0, 2, 1, 3, 4)).astype(BF16)            # [L,128,2,8,128]

        pc["w_e1"] = np.ascontiguousarray(
            w_e1[:, :, :, hcols].reshape(L, E, DC, 128, CH, 128)
            .transpose(0, 1, 3, 2, 4, 5)).astype(BF16)         # [L,E,128,8,4,128]
        pc["b_e1"] = np.ascontiguousarray(
            g["e_b1"][:, :, hcols].reshape(L, E, CH, 128)
            .transpose(0, 1, 3, 2)).astype(f32)                # [L,E,128,4]
        pc["w_e2"] = np.ascontiguousarray(
            g["e_w2"][:, :, hcols].reshape(L, E, CH, 128, DC, 128)
            .transpose(0, 1, 3, 2, 4, 5)).astype(BF16)         # [L,E,128,4,8,128]
        percore.append(pc)

    flags = {}
    return sh, percore, flags


# --------------------------------------------------------------------------
# Device kernel builder
# --------------------------------------------------------------------------

def _build():
    import concourse.mybir as mybir
    import concourse.tile as tile
    from concourse import bacc

    F32 = mybir.dt.float32
    BF = mybir.dt.bfloat16
    AF = mybir.ActivationFunctionType
    OP = mybir.AluOpType
    AX = mybir.AxisListType
    RG = [list(range(NCORES))]

    nc = bacc.Bacc("TRN2", target_bir_lowering=False, debug=False,
                   num_devices=NCORES)

    def din(name, shape, dt=BF):
        return nc.dram_tensor(name, list(shape), dt, kind="ExternalInput")

    t_x0 = din("x0", [128, DC], F32)
    t_w_in = din("w_in", [L, 128, DC, CIN, 128])
    t_b_in = din("b_in", [L, 128, CIN], F32)
    t_cpk = din("cpk", [L, 128, CXM, 3], F32)
    t_w_dtbc = din("w_dtbc", [L, 128, CXM, 3 * NS])
    t_b_dtbc = din("b_dtbc", [L, NS, 3], F32)
    t_w_s2i = din("w_s2i", [L, NS + 1, 256])
    t_w_out = din("w_out", [L, 128, CXM, DC, 128])
    t_b_out = din("b_out", [L, 128, DC], F32)
    t_w_gate = din("w_gate", [L, 128, DC, E])
    t_b_gate = din("b_gate", [L, 1, E], F32)
    t_w_e1 = din("w_e1", [L, E, 128, DC, CH, 128])
    t_b_e1 = din("b_e1", [L, E, 128, CH], F32)
    t_w_e2 = din("w_e2", [L, E, 128, CH, DC, 128])
    t_b_e2 = din("b_e2", [L, 128, E, DC], F32)
    t_w_hd = din("w_hd", [128, DC, NCLS])
    t_b_hd = din("b_hd", [1, NCLS], F32)
    t_out = nc.dram_tensor("out", [1, NCLS], F32, kind="ExternalOutput")

    with tile.TileContext(nc) as tc:
        with tc.tile_pool(name="consts", bufs=1) as consts, \
             tc.tile_pool(name="wi", bufs=2) as wip, \
             tc.tile_pool(name="wsm", bufs=2) as wsm, \
             tc.tile_pool(name="wo", bufs=2) as wop, \
             tc.tile_pool(name="we1", bufs=5) as we1p, \
             tc.tile_pool(name="we2", bufs=5) as we2p, \
             tc.tile_pool(name="bia", bufs=2) as biap, \
             tc.tile_pool(name="act", bufs=2) as actp, \
             tc.tile_pool(name="ps", bufs=1, space="PSUM") as psp, \
             tc.tile_pool(name="ard", bufs=4, space="DRAM") as ardp:

            ones_p = consts.tile([128, 1], BF)      # partition-sum lhsT
            nc.vector.memset(ones_p[:], 1.0)
            ones_pf = consts.tile([128, 1], F32)    # f32 partition-sum lhsT
            nc.vector.memset(ones_pf[:], 1.0)
            ones_b = consts.tile([1, 128], F32)     # broadcast lhsT (K=1)
            nc.vector.memset(ones_b[:], 1.0)

            _cregs = {}

            def creg(val, p=128):
                key = (val, p)
                if key not in _cregs:
                    ct = consts.tile([p, 1], F32, tag=f"c{len(_cregs)}")
                    nc.vector.memset(ct[:], val)
                    _cregs[key] = ct
                return _cregs[key][:]

            x_sb = consts.tile([128, DC], F32, tag="x")
            nc.sync.dma_start(out=x_sb[:], in_=t_x0.ap())

            # warm up the CC channels so the first real AllReduce is cheap
            warm = consts.tile([1, 1], F32, tag="warm")
            nc.vector.memset(warm[:], 0.0)
            wmd = ardp.tile([1, 1], F32, tag="warmd")
            nc.sync.dma_start(out=wmd[:], in_=warm[:])
            nc.gpsimd.collective_compute(
                "AllReduce", OP.add, replica_groups=RG,
                ins=[wmd[:]], outs=[wmd[:]])

            def rmsnorm(src, tag):
                """replicated rmsnorm of the [128, 8] vector -> bf16"""
                sq = actp.tile([128, DC], BF, tag=tag + "sq")
                nc.vector.tensor_mul(sq[:], src, src)
                pssum = psp.tile([128, DC], F32, tag="pmini")
                nc.tensor.matmul(pssum[0:1, :], ones_p[:], sq[:],
                                 start=True, stop=True)
                rs = actp.tile([1, 1], F32, tag=tag + "rs")
                nc.vector.tensor_reduce(out=rs[:], in_=pssum[0:1, :],
                                        axis=AX.X, op=OP.add)
                psb = psp.tile([128, DC], F32, tag="pmini")
                nc.tensor.matmul(psb[:, 0:1], ones_b[:], rs[:],
                                 start=True, stop=True)
                std = actp.tile([128, 1], F32, tag=tag + "std")
                nc.scalar.activation(std[:], psb[:, 0:1], AF.Sqrt,
                                     bias=creg(1e-6), scale=creg(1.0 / D))
                rinv = actp.tile([128, 1], F32, tag=tag + "ri")
                nc.vector.reciprocal(rinv[:], std[:])
                xn = actp.tile([128, DC], BF, tag=tag)
                nc.vector.tensor_mul(xn[:], src,
                                     rinv[:].broadcast_to([128, DC]))
                return xn

            for l in range(L):
                # ---------- mixer ----------
                xn1 = rmsnorm(x_sb[:], "xn1")

                wi = wip.tile([128, DC, CIN, 128], BF, tag="wi")
                nc.sync.dma_start(out=wi[:], in_=t_w_in.ap()[l])
                bi = biap.tile([128, CIN], F32, tag="bi")
                nc.sync.dma_start(out=bi[:], in_=t_b_in.ap()[l])
                cpk = biap.tile([128, CXM, 3], F32, tag="cpk")
                nc.sync.dma_start(out=cpk[:], in_=t_cpk.ap()[l])

                pin = psp.tile([128, CIN], F32, tag="pin")
                for j in range(CIN):
                    for k in range(DC):
                        nc.tensor.matmul(pin[:, j:j + 1], wi[:, k, j, :],
                                         xn1[:, k:k + 1], start=(k == 0),
                                         stop=(k == DC - 1))

                # conv tap at t=0 + silu on xm cols; sigmoid on gate cols
                xmp = actp.tile([128, CXM], F32, tag="xmp")
                nc.vector.tensor_add(xmp[:], pin[:, 0:CXM], bi[:, 0:CXM])
                nc.vector.tensor_mul(xmp[:], xmp[:], cpk[:, :, 0])
                nc.vector.tensor_add(xmp[:], xmp[:], cpk[:, :, 1])
                sgm = actp.tile([128, CXM], F32, tag="sgm")
                nc.scalar.activation(sgm[:], xmp[:], AF.Sigmoid)
                xm = actp.tile([128, CXM], F32, tag="xm")
                nc.vector.tensor_mul(xm[:], xmp[:], sgm[:])
                xmb = actp.tile([128, CXM], BF, tag="xmb")
                nc.scalar.copy(xmb[:], xm[:])
                gt = actp.tile([128, CXM], F32, tag="gt")
                nc.vector.tensor_add(gt[:], pin[:, CXM:CIN], bi[:, CXM:CIN])
                gsig = actp.tile([128, CXM], F32, tag="gsig")
                nc.scalar.activation(gsig[:], gt[:], AF.Sigmoid)

                # dt/B/C partial projections over this core's xm slice
                wd = wsm.tile([128, CXM, 3 * NS], BF, tag="wd")
                nc.sync.dma_start(out=wd[:], in_=t_w_dtbc.ap()[l])
                pd = psp.tile([128, 2], F32, tag="pd")
                for k in range(CXM):
                    nc.tensor.matmul(pd[:, 0:1], wd[:, k, 0:128],
                                     xmb[:, k:k + 1], start=(k == 0),
                                     stop=(k == CXM - 1))
                    nc.tensor.matmul(pd[0:NS, 1:2], wd[:, k, 128:192],
                                     xmb[:, k:k + 1], start=(k == 0),
                                     stop=(k == CXM - 1))

                ar1s = actp.tile([128, 2], F32, tag="ar1s")
                nc.scalar.copy(ar1s[:, 0:1], pd[:, 0:1])
                nc.scalar.copy(ar1s[0:NS, 1:2], pd[0:NS, 1:2])
                ar1 = ardp.tile([3 * NS, 1], F32, tag="ar1")
                nc.sync.dma_start(out=ar1[0:128, :], in_=ar1s[:, 0:1])
                nc.sync.dma_start(out=ar1[128:192, :], in_=ar1s[0:NS, 1:2])
                nc.gpsimd.collective_compute(
                    "AllReduce", OP.add, replica_groups=RG,
                    ins=[ar1[:]], outs=[ar1[:]])
                dtbc = actp.tile([NS, 3], F32, tag="dtbc")
                nc.sync.dma_start(
                    out=dtbc[:],
                    in_=ar1[:].rearrange("(c s) one -> s (c one)", c=3))
                bdt = biap.tile([NS, 3], F32, tag="bdt")
                nc.sync.dma_start(out=bdt[:], in_=t_b_dtbc.ap()[l])
                nc.vector.tensor_add(dtbc[:], dtbc[:], bdt[:])

                # SSM at t=0: state = dt*B ; y = C*state ; LN over 64
                dt_t = actp.tile([NS, 1], F32, tag="dt")
                nc.scalar.activation(dt_t[:], dtbc[:, 0:1], AF.Sigmoid)
                y_t = actp.tile([NS, 2], F32, tag="y")
                nc.vector.tensor_mul(y_t[:, 0:1], dt_t[:], dtbc[:, 1:2])
                nc.vector.tensor_mul(y_t[:, 0:1], y_t[:, 0:1], dtbc[:, 2:3])
                nc.vector.tensor_mul(y_t[:, 1:2], y_t[:, 0:1], y_t[:, 0:1])
                psl = psp.tile([128, 2], F32, tag="pmini2")
                nc.tensor.matmul(psl[0:1, :], ones_pf[0:NS, :], y_t[:],
                                 start=True, stop=True)
                mu = actp.tile([1, 2], F32, tag="mu")   # [mean, mean-of-sq]
                nc.vector.tensor_scalar(out=mu[:], in0=psl[0:1, :],
                                        scalar1=1.0 / NS, scalar2=None,
                                        op0=OP.mult)
                var = actp.tile([1, 1], F32, tag="var")
                nc.vector.tensor_mul(var[:], mu[:, 0:1], mu[:, 0:1])
                nc.vector.tensor_sub(var[:], mu[:, 1:2], var[:])
                stdl = actp.tile([1, 1], F32, tag="stdl")
                nc.scalar.activation(stdl[:], var[:], AF.Sqrt,
                                     bias=creg(1e-5, 1))
                ri = actp.tile([1, 2], F32, tag="ri2")  # [rstd, mean]
                nc.vector.reciprocal(ri[:, 0:1], stdl[:])
                nc.scalar.copy(ri[:, 1:2], mu[:, 0:1])
                psb2 = psp.tile([128, 2], F32, tag="pmini2")
                nc.tensor.matmul(psb2[0:NS, :], ones_b[:, 0:NS], ri[:],
                                 start=True, stop=True)
                yn = actp.tile([NS + 1, 1], BF, tag="yn")
                ytmp = actp.tile([NS, 1], F32, tag="ytmp")
                nc.vector.tensor_sub(ytmp[:], y_t[:, 0:1], psb2[0:NS, 1:2])
                nc.vector.tensor_mul(ytmp[:], ytmp[:], psb2[0:NS, 0:1])
                nc.scalar.copy(yn[0:NS, :], ytmp[:])
                nc.vector.memset(yn[NS:NS + 1, :], 1.0)

                # s2i (+bias row) + D*xm, gated; then out-proj partial
                ws2 = wsm.tile([NS + 1, 256], BF, tag="ws2")
                nc.sync.dma_start(out=ws2[:], in_=t_w_s2i.ap()[l])
                pz = psp.tile([128, CXM], F32, tag="pd")
                for j in range(CXM):
                    nc.tensor.matmul(pz[:, j:j + 1],
                                     ws2[:, j * 128:(j + 1) * 128],
                                     yn[:], start=True, stop=True)
                z = actp.tile([128, CXM], F32, tag="z")
                nc.vector.tensor_mul(z[:], xm[:], cpk[:, :, 2])
                nc.vector.tensor_add(z[:], z[:], pz[:])
                nc.vector.tensor_mul(z[:], z[:], gsig[:])
                zb = actp.tile([128, CXM], BF, tag="zb")
                nc.scalar.copy(zb[:], z[:])

                wo = wop.tile([128, CXM, DC, 128], BF, tag="wo")
                nc.sync.dma_start(out=wo[:], in_=t_w_out.ap()[l])
                po = psp.tile([128, DC], F32, tag="po")
                for m in range(DC):
                    for k in range(CXM):
                        nc.tensor.matmul(po[:, m:m + 1], wo[:, k, m, :],
                                         zb[:, k:k + 1], start=(k == 0),
                                         stop=(k == CXM - 1))
                ar2s = actp.tile([128, DC], F32, tag="ar2s")
                nc.scalar.copy(ar2s[:], po[:])
                ar2 = ardp.tile([128, DC], F32, tag="ar2")
                nc.sync.dma_start(out=ar2[:], in_=ar2s[:])
                nc.gpsimd.collective_compute(
                    "AllReduce", OP.add, replica_groups=RG,
                    ins=[ar2[:]], outs=[ar2[:]])
                mix = actp.tile([128, DC], F32, tag="mix")
                nc.sync.dma_start(out=mix[:], in_=ar2[:])
                bo = biap.tile([128, DC], F32, tag="bo")
                nc.sync.dma_start(out=bo[:], in_=t_b_out.ap()[l])
                nc.vector.tensor_add(mix[:], mix[:], bo[:])
                nc.vector.tensor_add(x_sb[:], x_sb[:], mix[:])

                # ---------- MoE ----------
                xn2 = rmsnorm(x_sb[:], "xn2")

                wg = wsm.tile([128, DC, E], BF, tag="wg")
                nc.sync.dma_start(out=wg[:], in_=t_w_gate.ap()[l])
                bg = biap.tile([1, E], F32, tag="bg")
                nc.sync.dma_start(out=bg[:], in_=t_b_gate.ap()[l])
                pg = psp.tile([128, E], F32, tag="pmini")
                for k in range(DC):
                    nc.tensor.matmul(pg[0:1, :], xn2[:, k:k + 1], wg[:, k, :],
                                     start=(k == 0), stop=(k == DC - 1))
                lg = actp.tile([1, E], F32, tag="lg")
                nc.vector.tensor_add(lg[:], pg[0:1, :], bg[:])
                m1 = actp.tile([1, 1], F32, tag="m1")
                nc.vector.tensor_reduce(out=m1[:], in_=lg[:], axis=AX.X,
                                        op=OP.max)
                mask1 = actp.tile([1, E], F32, tag="mask1")
                nc.vector.tensor_tensor(out=mask1[:], in0=lg[:],
                                        in1=m1[:].broadcast_to([1, E]),
                                        op=OP.is_ge)
                l2 = actp.tile([1, E], F32, tag="l2")
                nc.vector.scalar_tensor_tensor(
                    out=l2[:], in0=mask1[:], scalar=creg(-1e9, 1), in1=lg[:],
                    op0=OP.mult, op1=OP.add)
                m2 = actp.tile([1, 1], F32, tag="m2")
                nc.vector.tensor_reduce(out=m2[:], in_=l2[:], axis=AX.X,
                                        op=OP.max)
                dgap = actp.tile([1, 1], F32, tag="dgap")
                nc.vector.tensor_sub(dgap[:], m1[:], m2[:])
                p1 = actp.tile([1, 1], F32, tag="p1")
                nc.scalar.activation(p1[:], dgap[:], AF.Sigmoid)
                p2 = actp.tile([1, 1], F32, tag="p2")
                nc.vector.tensor_scalar(out=p2[:], in0=p1[:], scalar1=-1.0,
                                        scalar2=1.0, op0=OP.mult, op1=OP.add)
                mask2 = actp.tile([1, E], F32, tag="mask2")
                nc.vector.tensor_tensor(out=mask2[:], in0=l2[:],
                                        in1=m2[:].broadcast_to([1, E]),
                                        op=OP.is_ge)
                wsel = actp.tile([1, E], F32, tag="wsel")
                nc.vector.tensor_mul(wsel[:], mask1[:],
                                     p1[:].broadcast_to([1, E]))
                wsel2 = actp.tile([1, E], F32, tag="wsel2")
                nc.vector.tensor_mul(wsel2[:], mask2[:],
                                     p2[:].broadcast_to([1, E]))
                nc.vector.tensor_add(wsel[:], wsel[:], wsel2[:])
                pgb = psp.tile([128, E], F32, tag="pmini")
                nc.tensor.matmul(pgb[:], ones_b[:], wsel[:],
                                 start=True, stop=True)
                wbc = actp.tile([128, E], F32, tag="wbc")
                nc.scalar.copy(wbc[:], pgb[:])

                macc = actp.tile([128, DC], F32, tag="macc")
                b2w = actp.tile([128, DC], F32, tag="b2w")
                be2 = biap.tile([128, E, DC], F32, tag="be2")
                nc.sync.dma_start(out=be2[:], in_=t_b_e2.ap()[l])
                for e in range(E):
                    w1 = we1p.tile([128, DC, CH, 128], BF, tag="we1")
                    nc.sync.dma_start(out=w1[:], in_=t_w_e1.ap()[l, e])
                    be1 = biap.tile([128, CH], F32, tag="be1")
                    nc.sync.dma_start(out=be1[:], in_=t_b_e1.ap()[l, e])
                    ph = psp.tile([128, CH], F32, tag="ph", bufs=1)
                    for j in range(CH):
                        for k in range(DC):
                            nc.tensor.matmul(ph[:, j:j + 1], w1[:, k, j, :],
                                             xn2[:, k:k + 1], start=(k == 0),
                                             stop=(k == DC - 1))
                    hsum = actp.tile([128, CH], F32, tag="hsum")
                    nc.vector.tensor_add(hsum[:], ph[:], be1[:])
                    hg = actp.tile([128, CH], BF, tag="hg")
                    nc.scalar.activation(hg[:], hsum[:], AF.Gelu)

                    w2 = we2p.tile([128, CH, DC, 128], BF, tag="we2")
                    nc.sync.dma_start(out=w2[:], in_=t_w_e2.ap()[l, e])
                    pe2 = psp.tile([128, DC], F32, tag="pe2", bufs=2)
                    for m in range(DC):
                        for k in range(CH):
                            nc.tensor.matmul(pe2[:, m:m + 1], w2[:, k, m, :],
                                             hg[:, k:k + 1], start=(k == 0),
                                             stop=(k == CH - 1))
                    if e == 0:
                        nc.vector.scalar_tensor_tensor(
                            out=macc[:], in0=pe2[:], scalar=wbc[:, 0:1],
                            in1=x_sb[:], op0=OP.mult, op1=OP.bypass)
                        nc.vector.scalar_tensor_tensor(
                            out=b2w[:], in0=be2[:, 0, :], scalar=wbc[:, 0:1],
                            in1=be2[:, 0, :], op0=OP.mult, op1=OP.bypass)
                    else:
                        nc.vector.scalar_tensor_tensor(
                            out=macc[:], in0=pe2[:], scalar=wbc[:, e:e + 1],
                            in1=macc[:], op0=OP.mult, op1=OP.add)
                        nc.vector.scalar_tensor_tensor(
                            out=b2w[:], in0=be2[:, e, :],
                            scalar=wbc[:, e:e + 1],
                            in1=b2w[:], op0=OP.mult, op1=OP.add)

                ar3 = ardp.tile([128, DC], F32, tag="ar3")
                nc.sync.dma_start(out=ar3[:], in_=macc[:])
                nc.gpsimd.collective_compute(
                    "AllReduce", OP.add, replica_groups=RG,
                    ins=[ar3[:]], outs=[ar3[:]])
                moe = actp.tile([128, DC], F32, tag="moe")
                nc.sync.dma_start(out=moe[:], in_=ar3[:])
                nc.vector.tensor_add(moe[:], moe[:], b2w[:])
                nc.vector.tensor_add(x_sb[:], x_sb[:], moe[:])

            # ---------- head ----------
            xf = rmsnorm(x_sb[:], "xf")
            whd = consts.tile([128, DC, NCLS], BF, tag="whd")
            nc.sync.dma_start(out=whd[:], in_=t_w_hd.ap())
            bhd = consts.tile([1, NCLS], F32, tag="bhd")
            nc.sync.dma_start(out=bhd[:], in_=t_b_hd.ap())
            phd = psp.tile([128, NCLS], F32, tag="pmini")
            for k in range(DC):
                nc.tensor.matmul(phd[0:1, :], xf[:, k:k + 1], whd[:, k, :],
                                 start=(k == 0), stop=(k == DC - 1))
            osb = actp.tile([1, NCLS], F32, tag="osb")
            nc.vector.tensor_add(osb[:], phd[0:1, :], bhd[:])
            nc.sync.dma_start(out=t_out.ap(), in_=osb[:])

    nc.compile()
    return nc


def get_nc(flags):
    if "nc" not in _CACHE:
        _CACHE["nc"] = _build()
    return _CACHE["nc"]


def kernel(**inputs):
    from concourse.bass_utils import run_bass_kernel_spmd
    sh, percore, flags = _prep(inputs)
    nc = get_nc(flags)
    in_maps = [{**sh, **pc} for pc in percore]
    res = run_bass_kernel_spmd(nc, in_maps, core_ids=list(range(NCORES)))
    row = np.asarray(res.results[0]["out"], np.float32).reshape(NCLS)
    return np.ascontiguousarray(
        np.broadcast_to(row[None, :], (B, NCLS))).astype(np.float32)


# revision 26
# speedup vs baseline: 1.0157x; 1.0157x over previous
"""Trainium2 Bass kernel for nn_MixtureOfMambaModel.

Exact graph-level optimization: the classifier head reads x[:, 0] (the cls
token), and every sequence-mixing op in the model is causal (depthwise conv
with left-only padding, forward SSM scan) or per-token (norms, MoE, router).
Token 0 therefore never observes tokens 1..97, and its initial value is
cls_token + modality_embed[:,3] + pos_embed[:,0] — independent of the video /
audio / question inputs. The model output is a function of the weights only,
identical across the batch. The kernel computes that single-token forward
pass exactly, on device, and broadcasts the result to all 16 batch rows.

Device strategy (8 NeuronCores, tensor-parallel single-token forward):
  - All big projections are split 8 ways: in_proj / expert-w1 by output
    columns, out_proj / expert-w2 by contraction rows. The [1024] activation
    vector is replicated as a [128, 8] tile on every core.
  - Three 4KB AllReduces per layer stitch the partials together:
    dt/B/C projections [192], mixer output [1024], weighted MoE output
    [1024]. Collectives run on internal DRAM tiles (CCE fp32 add).
  - Small/serial pieces (rmsnorm, SSM step at t=0, layernorm over 64,
    router top-2, conv tap) are replicated on every core — they are a few
    hundred elements each.
  - Matmuls run stationary-weight with a 1-column moving operand (the
    token), bf16 in / fp32 PSUM accumulation. Biases and norm weights are
    folded host-side exactly as in the dense formulation.
"""

import numpy as np
import ml_dtypes

# ---- model dims (hardcoded per spec) ----
B = 16
D = 1024
INNER = 2048
NS = 64
HID = 4096
E = 4
L = 4
NCLS = 13
DC = D // 128                # 8 chunks of the model dim
NCORES = 8
CIN = 2 * INNER // NCORES // 128   # in-proj col chunks per core (4)
CXM = INNER // NCORES // 128       # xm col chunks per core (2)
CH = HID // NCORES // 128          # expert hidden chunks per core (4)

BF16 = ml_dtypes.bfloat16

_CACHE = {}


# --------------------------------------------------------------------------
# Host-side preparation: slicing / layout / constant folding on weights.
# --------------------------------------------------------------------------

def _prep(inputs):
    f32 = np.float32
    g = {k: np.asarray(v) for k, v in inputs.items()}

    # token-0 initial value: cls + modality_embed[3] + pos_embed[0]
    x0 = (np.asarray(g["cls_token"][0, 0], f32)
          + np.asarray(g["modality_embed"][0, 3], f32)
          + np.asarray(g["pos_embed"][0, 0], f32))            # [1024]

    sh = {}
    sh["x0"] = np.ascontiguousarray(x0.reshape(DC, 128).T).astype(f32)  # [128, 8]

    w_in = (g["in_w"] * g["norm1_w"][:, :, None]).astype(f32)  # [L,1024,4096]
    w_gate = (g["gate_w"] * g["norm2_w"][:, :, None]).astype(f32)
    w_e1 = (g["e_w1"] * g["norm2_w"][:, None, :, None]).astype(f32)
    w_hd = (g["head_w"] * g["fnorm_w"][:, None]).astype(f32)   # [1024, 13]

    # replicated (shared) tensors
    sh["w_gate"] = np.ascontiguousarray(
        w_gate.reshape(L, DC, 128, E).transpose(0, 2, 1, 3)).astype(BF16)
    sh["b_gate"] = g["gate_b"].reshape(L, 1, E).astype(f32)
    sh["b_dtbc"] = np.ascontiguousarray(
        np.stack([g["dt_b"], g["Bp_b"], g["Cp_b"]], axis=2)).astype(f32)
    sh["b_out"] = np.ascontiguousarray(
        g["out_b"].reshape(L, DC, 128).transpose(0, 2, 1)).astype(f32)
    sh["b_e2"] = np.ascontiguousarray(
        g["e_b2"].reshape(L, E, DC, 128).transpose(0, 3, 1, 2)).astype(f32)
    sh["w_hd"] = np.ascontiguousarray(
        w_hd.reshape(DC, 128, NCLS).transpose(1, 0, 2)).astype(BF16)
    sh["b_hd"] = g["head_b"].reshape(1, NCLS).astype(f32)

    percore = []
    for c in range(NCORES):
        pc = {}
        mcols = slice(c * 256, (c + 1) * 256)                  # xm cols
        gcols = slice(INNER + c * 256, INNER + (c + 1) * 256)  # gate cols
        hcols = slice(c * 512, (c + 1) * 512)                  # hidden cols

        wi = np.concatenate([w_in[:, :, mcols], w_in[:, :, gcols]], axis=2)
        # [L, 1024, 512] -> [L, 128p, 8k, 4j, 128m]
        pc["w_in"] = np.ascontiguousarray(
            wi.reshape(L, DC, 128, CIN, 128).transpose(0, 2, 1, 3, 4)
        ).astype(BF16)
        bi = np.concatenate([g["in_b"][:, mcols], g["in_b"][:, gcols]], 1)
        pc["b_in"] = np.ascontiguousarray(
            bi.reshape(L, CIN, 128).transpose(0, 2, 1)).astype(f32)

        cpk = np.zeros((L, 128, CXM, 3), f32)
        cpk[:, :, :, 0] = g["conv_w"][:, mcols, 0, 2].reshape(
            L, CXM, 128).transpose(0, 2, 1)
        cpk[:, :, :, 1] = g["conv_b"][:, mcols].reshape(
            L, CXM, 128).transpose(0, 2, 1)
        cpk[:, :, :, 2] = g["D_param"][:, mcols].reshape(
            L, CXM, 128).transpose(0, 2, 1)
        pc["cpk"] = cpk

        wd = np.concatenate([g["dt_w"], g["Bp_w"], g["Cp_w"]], 2)[:, mcols]
        pc["w_dtbc"] = np.ascontiguousarray(
            wd.reshape(L, CXM, 128, 3 * NS).transpose(0, 2, 1, 3)
        ).astype(BF16)                                         # [L,128,2,192]

        s2 = np.concatenate(
            [g["s2i_w"][:, :, mcols], g["s2i_b"][:, None, mcols]], 1)
        pc["w_s2i"] = np.ascontiguousarray(s2).astype(BF16)    # [L, 65, 256]

        pc["w_out"] = np.ascontiguousarray(
            g["out_w"][:, mcols].reshape(L, CXM, 128, DC, 128)
            .transpose(0, 2, 1, 3, 4)).astype(BF16)            # [L,128,2,8,128]

        pc["w_e1"] = np.ascontiguousarray(
            w_e1[:, :, :, hcols].reshape(L, E, DC, 128, CH, 128)
            .transpose(0, 1, 3, 2, 4, 5)).astype(BF16)         # [L,E,128,8,4,128]
        pc["b_e1"] = np.ascontiguousarray(
            g["e_b1"][:, :, hcols].reshape(L, E, CH, 128)
            .transpose(0, 1, 3, 2)).astype(f32)                # [L,E,128,4]
        pc["w_e2"] = np.ascontiguousarray(
            g["e_w2"][:, :, hcols].reshape(L, E, CH, 128, DC, 128)
            .transpose(0, 1, 3, 2, 4, 5)).astype(BF16)         # [L,E,128,4,8,128]
        percore.append(pc)

    flags = {}
    return sh, percore, flags


# --------------------------------------------------------------------------
# Device kernel builder
# --------------------------------------------------------------------------

def _build():
    import concourse.mybir as mybir
    import concourse.tile as tile
    from concourse import bacc

    F32 = mybir.dt.float32
    BF = mybir.dt.bfloat16
    AF = mybir.ActivationFunctionType
    OP = mybir.AluOpType
    AX = mybir.AxisListType
    RG = [list(range(NCORES))]

    nc = bacc.Bacc("TRN2", target_bir_lowering=False, debug=False,
                   num_devices=NCORES)

    def din(name, shape, dt=BF):
        return nc.dram_tensor(name, list(shape), dt, kind="ExternalInput")

    t_x0 = din("x0", [128, DC], F32)
    t_w_in = din("w_in", [L, 128, DC, CIN, 128])
    t_b_in = din("b_in", [L, 128, CIN], F32)
    t_cpk = din("cpk", [L, 128, CXM, 3], F32)
    t_w_dtbc = din("w_dtbc", [L, 128, CXM, 3 * NS])
    t_b_dtbc = din("b_dtbc", [L, NS, 3], F32)
    t_w_s2i = din("w_s2i", [L, NS + 1, 256])
    t_w_out = din("w_out", [L, 128, CXM, DC, 128])
    t_b_out = din("b_out", [L, 128, DC], F32)
    t_w_gate = din("w_gate", [L, 128, DC, E])
    t_b_gate = din("b_gate", [L, 1, E], F32)
    t_w_e1 = din("w_e1", [L, E, 128, DC, CH, 128])
    t_b_e1 = din("b_e1", [L, E, 128, CH], F32)
    t_w_e2 = din("w_e2", [L, E, 128, CH, DC, 128])
    t_b_e2 = din("b_e2", [L, 128, E, DC], F32)
    t_w_hd = din("w_hd", [128, DC, NCLS])
    t_b_hd = din("b_hd", [1, NCLS], F32)
    t_out = nc.dram_tensor("out", [1, NCLS], F32, kind="ExternalOutput")

    with tile.TileContext(nc) as tc:
        with tc.tile_pool(name="consts", bufs=1) as consts, \
             tc.tile_pool(name="wi", bufs=2) as wip, \
             tc.tile_pool(name="wsm", bufs=2) as wsm, \
             tc.tile_pool(name="wo", bufs=2) as wop, \
             tc.tile_pool(name="we1", bufs=5) as we1p, \
             tc.tile_pool(name="we2", bufs=5) as we2p, \
             tc.tile_pool(name="bia", bufs=2) as biap, \
             tc.tile_pool(name="act", bufs=2) as actp, \
             tc.tile_pool(name="ps", bufs=1, space="PSUM") as psp, \
             tc.tile_pool(name="ard", bufs=4, space="DRAM") as ardp:

            ones_p = consts.tile([128, 1], BF)      # partition-sum lhsT
            nc.vector.memset(ones_p[:], 1.0)
            ones_pf = consts.tile([128, 1], F32)    # f32 partition-sum lhsT
            nc.vector.memset(ones_pf[:], 1.0)
            ones_b = consts.tile([1, 128], F32)     # broadcast lhsT (K=1)
            nc.vector.memset(ones_b[:], 1.0)

            _cregs = {}

            def creg(val, p=128):
                key = (val, p)
                if key not in _cregs:
                    ct = consts.tile([p, 1], F32, tag=f"c{len(_cregs)}")
                    nc.vector.memset(ct[:], val)
                    _cregs[key] = ct
                return _cregs[key][:]

            x_sb = consts.tile([128, DC], F32, tag="x")
            nc.sync.dma_start(out=x_sb[:], in_=t_x0.ap())

            def rmsnorm(src, tag):
                """replicated rmsnorm of the [128, 8] vector -> bf16"""
                sq = actp.tile([128, DC], BF, tag=tag + "sq")
                nc.vector.tensor_mul(sq[:], src, src)
                pssum = psp.tile([128, DC], F32, tag="pmini")
                nc.tensor.matmul(pssum[0:1, :], ones_p[:], sq[:],
                                 start=True, stop=True)
                rs = actp.tile([1, 1], F32, tag=tag + "rs")
                nc.vector.tensor_reduce(out=rs[:], in_=pssum[0:1, :],
                                        axis=AX.X, op=OP.add)
                psb = psp.tile([128, DC], F32, tag="pmini")
                nc.tensor.matmul(psb[:, 0:1], ones_b[:], rs[:],
                                 start=True, stop=True)
                std = actp.tile([128, 1], F32, tag=tag + "std")
                nc.scalar.activation(std[:], psb[:, 0:1], AF.Sqrt,
                                     bias=creg(1e-6), scale=creg(1.0 / D))
                rinv = actp.tile([128, 1], F32, tag=tag + "ri")
                nc.vector.reciprocal(rinv[:], std[:])
                xn = actp.tile([128, DC], BF, tag=tag)
                nc.vector.tensor_mul(xn[:], src,
                                     rinv[:].broadcast_to([128, DC]))
                return xn

            for l in range(L):
                # ---------- mixer ----------
                xn1 = rmsnorm(x_sb[:], "xn1")

                wi = wip.tile([128, DC, CIN, 128], BF, tag="wi")
                nc.sync.dma_start(out=wi[:], in_=t_w_in.ap()[l])
                bi = biap.tile([128, CIN], F32, tag="bi")
                nc.sync.dma_start(out=bi[:], in_=t_b_in.ap()[l])
                cpk = biap.tile([128, CXM, 3], F32, tag="cpk")
                nc.sync.dma_start(out=cpk[:], in_=t_cpk.ap()[l])

                pin = psp.tile([128, CIN], F32, tag="pin")
                for j in range(CIN):
                    for k in range(DC):
                        nc.tensor.matmul(pin[:, j:j + 1], wi[:, k, j, :],
                                         xn1[:, k:k + 1], start=(k == 0),
                                         stop=(k == DC - 1))

                # conv tap at t=0 + silu on xm cols; sigmoid on gate cols
                xmp = actp.tile([128, CXM], F32, tag="xmp")
                nc.vector.tensor_add(xmp[:], pin[:, 0:CXM], bi[:, 0:CXM])
                nc.vector.tensor_mul(xmp[:], xmp[:], cpk[:, :, 0])
                nc.vector.tensor_add(xmp[:], xmp[:], cpk[:, :, 1])
                sgm = actp.tile([128, CXM], F32, tag="sgm")
                nc.scalar.activation(sgm[:], xmp[:], AF.Sigmoid)
                xm = actp.tile([128, CXM], F32, tag="xm")
                nc.vector.tensor_mul(xm[:], xmp[:], sgm[:])
                xmb = actp.tile([128, CXM], BF, tag="xmb")
                nc.scalar.copy(xmb[:], xm[:])
                gt = actp.tile([128, CXM], F32, tag="gt")
                nc.vector.tensor_add(gt[:], pin[:, CXM:CIN], bi[:, CXM:CIN])
                gsig = actp.tile([128, CXM], F32, tag="gsig")
                nc.scalar.activation(gsig[:], gt[:], AF.Sigmoid)

                # dt/B/C partial projections over this core's xm slice
                wd = wsm.tile([128, CXM, 3 * NS], BF, tag="wd")
                nc.sync.dma_start(out=wd[:], in_=t_w_dtbc.ap()[l])
                pd = psp.tile([128, 2], F32, tag="pd")
                for k in range(CXM):
                    nc.tensor.matmul(pd[:, 0:1], wd[:, k, 0:128],
                                     xmb[:, k:k + 1], start=(k == 0),
                                     stop=(k == CXM - 1))
                    nc.tensor.matmul(pd[0:NS, 1:2], wd[:, k, 128:192],
                                     xmb[:, k:k + 1], start=(k == 0),
                                     stop=(k == CXM - 1))

                ar1s = actp.tile([128, 2], F32, tag="ar1s")
                nc.scalar.copy(ar1s[:, 0:1], pd[:, 0:1])
                nc.scalar.copy(ar1s[0:NS, 1:2], pd[0:NS, 1:2])
                ar1 = ardp.tile([3 * NS, 1], F32, tag="ar1")
                nc.sync.dma_start(out=ar1[0:128, :], in_=ar1s[:, 0:1])
                nc.sync.dma_start(out=ar1[128:192, :], in_=ar1s[0:NS, 1:2])
                nc.gpsimd.collective_compute(
                    "AllReduce", OP.add, replica_groups=RG,
                    ins=[ar1[:]], outs=[ar1[:]])
                dtbc = actp.tile([NS, 3], F32, tag="dtbc")
                nc.sync.dma_start(
                    out=dtbc[:],
                    in_=ar1[:].rearrange("(c s) one -> s (c one)", c=3))
                bdt = biap.tile([NS, 3], F32, tag="bdt")
                nc.sync.dma_start(out=bdt[:], in_=t_b_dtbc.ap()[l])
                nc.vector.tensor_add(dtbc[:], dtbc[:], bdt[:])

                # SSM at t=0: state = dt*B ; y = C*state ; LN over 64
                dt_t = actp.tile([NS, 1], F32, tag="dt")
                nc.scalar.activation(dt_t[:], dtbc[:, 0:1], AF.Sigmoid)
                y_t = actp.tile([NS, 2], F32, tag="y")
                nc.vector.tensor_mul(y_t[:, 0:1], dt_t[:], dtbc[:, 1:2])
                nc.vector.tensor_mul(y_t[:, 0:1], y_t[:, 0:1], dtbc[:, 2:3])
                nc.vector.tensor_mul(y_t[:, 1:2], y_t[:, 0:1], y_t[:, 0:1])
                psl = psp.tile([128, 2], F32, tag="pmini2")
                nc.tensor.matmul(psl[0:1, :], ones_pf[0:NS, :], y_t[:],
                                 start=True, stop=True)
                mu = actp.tile([1, 2], F32, tag="mu")   # [mean, mean-of-sq]
                nc.vector.tensor_scalar(out=mu[:], in0=psl[0:1, :],
                                        scalar1=1.0 / NS, scalar2=None,
                                        op0=OP.mult)
                var = actp.tile([1, 1], F32, tag="var")
                nc.vector.tensor_mul(var[:], mu[:, 0:1], mu[:, 0:1])
                nc.vector.tensor_sub(var[:], mu[:, 1:2], var[:])
                stdl = actp.tile([1, 1], F32, tag="stdl")
                nc.scalar.activation(stdl[:], var[:], AF.Sqrt,
                                     bias=creg(1e-5, 1))
                ri = actp.tile([1, 2], F32, tag="ri2")  # [rstd, mean]
                nc.vector.reciprocal(ri[:, 0:1], stdl[:])
                nc.scalar.copy(ri[:, 1:2], mu[:, 0:1])
                psb2 = psp.tile([128, 2], F32, tag="pmini2")
                nc.tensor.matmul(psb2[0:NS, :], ones_b[:, 0:NS], ri[:],
                                 start=True, stop=True)
                yn = actp.tile([NS + 1, 1], BF, tag="yn")
                ytmp = actp.tile([NS, 1], F32, tag="ytmp")
                nc.vector.tensor_sub(ytmp[:], y_t[:, 0:1], psb2[0:NS, 1:2])
                nc.vector.tensor_mul(ytmp[:], ytmp[:], psb2[0:NS, 0:1])
                nc.scalar.copy(yn[0:NS, :], ytmp[:])
                nc.vector.memset(yn[NS:NS + 1, :], 1.0)

                # s2i (+bias row) + D*xm, gated; then out-proj partial
                ws2 = wsm.tile([NS + 1, 256], BF, tag="ws2")
                nc.sync.dma_start(out=ws2[:], in_=t_w_s2i.ap()[l])
                pz = psp.tile([128, CXM], F32, tag="pd")
                for j in range(CXM):
                    nc.tensor.matmul(pz[:, j:j + 1],
                                     ws2[:, j * 128:(j + 1) * 128],
                                     yn[:], start=True, stop=True)
                z = actp.tile([128, CXM], F32, tag="z")
                nc.vector.tensor_mul(z[:], xm[:], cpk[:, :, 2])
                nc.vector.tensor_add(z[:], z[:], pz[:])
                nc.vector.tensor_mul(z[:], z[:], gsig[:])
                zb = actp.tile([128, CXM], BF, tag="zb")
                nc.scalar.copy(zb[:], z[:])

                wo = wop.tile([128, CXM, DC, 128], BF, tag="wo")
                nc.sync.dma_start(out=wo[:], in_=t_w_out.ap()[l])
                po = psp.tile([128, DC], F32, tag="po")
                for m in range(DC):
                    for k in range(CXM):
                        nc.tensor.matmul(po[:, m:m + 1], wo[:, k, m, :],
                                         zb[:, k:k + 1], start=(k == 0),
                                         stop=(k == CXM - 1))
                ar2s = actp.tile([128, DC], F32, tag="ar2s")
                nc.scalar.copy(ar2s[:], po[:])
                ar2 = ardp.tile([128, DC], F32, tag="ar2")
                nc.sync.dma_start(out=ar2[:], in_=ar2s[:])
                nc.gpsimd.collective_compute(
                    "AllReduce", OP.add, replica_groups=RG,
                    ins=[ar2[:]], outs=[ar2[:]])
                mix = actp.tile([128, DC], F32, tag="mix")
                nc.sync.dma_start(out=mix[:], in_=ar2[:])
                bo = biap.tile([128, DC], F32, tag="bo")
                nc.sync.dma_start(out=bo[:], in_=t_b_out.ap()[l])
                nc.vector.tensor_add(mix[:], mix[:], bo[:])
                nc.vector.tensor_add(x_sb[:], x_sb[:], mix[:])

                # ---------- MoE ----------
                xn2 = rmsnorm(x_sb[:], "xn2")

                wg = wsm.tile([128, DC, E], BF, tag="wg")
                nc.sync.dma_start(out=wg[:], in_=t_w_gate.ap()[l])
                bg = biap.tile([1, E], F32, tag="bg")
                nc.sync.dma_start(out=bg[:], in_=t_b_gate.ap()[l])
                pg = psp.tile([128, E], F32, tag="pmini")
                for k in range(DC):
                    nc.tensor.matmul(pg[0:1, :], xn2[:, k:k + 1], wg[:, k, :],
                                     start=(k == 0), stop=(k == DC - 1))
                lg = actp.tile([1, E], F32, tag="lg")
                nc.vector.tensor_add(lg[:], pg[0:1, :], bg[:])
                m1 = actp.tile([1, 1], F32, tag="m1")
                nc.vector.tensor_reduce(out=m1[:], in_=lg[:], axis=AX.X,
                                        op=OP.max)
                mask1 = actp.tile([1, E], F32, tag="mask1")
                nc.vector.tensor_tensor(out=mask1[:], in0=lg[:],
                                        in1=m1[:].broadcast_to([1, E]),
                                        op=OP.is_ge)
                l2 = actp.tile([1, E], F32, tag="l2")
                nc.vector.scalar_tensor_tensor(
                    out=l2[:], in0=mask1[:], scalar=creg(-1e9, 1), in1=lg[:],
                    op0=OP.mult, op1=OP.add)
                m2 = actp.tile([1, 1], F32, tag="m2")
                nc.vector.tensor_reduce(out=m2[:], in_=l2[:], axis=AX.X,
                                        op=OP.max)
                dgap = actp.tile([1, 1], F32, tag="dgap")
                nc.vector.tensor_sub(dgap[:], m1[:], m2[:])
                p1 = actp.tile([1, 1], F32, tag="p1")
                nc.scalar.activation(p1[:], dgap[:], AF.Sigmoid)
                p2 = actp.tile([1, 1], F32, tag="p2")
                nc.vector.tensor_scalar(out=p2[:], in0=p1[:], scalar1=-1.0,
                                        scalar2=1.0, op0=OP.mult, op1=OP.add)
                mask2 = actp.tile([1, E], F32, tag="mask2")
                nc.vector.tensor_tensor(out=mask2[:], in0=l2[:],
                                        in1=m2[:].broadcast_to([1, E]),
                                        op=OP.is_ge)
                wsel = actp.tile([1, E], F32, tag="wsel")
                nc.vector.tensor_mul(wsel[:], mask1[:],
                                     p1[:].broadcast_to([1, E]))
                wsel2 = actp.tile([1, E], F32, tag="wsel2")
                nc.vector.tensor_mul(wsel2[:], mask2[:],
                                     p2[:].broadcast_to([1, E]))
                nc.vector.tensor_add(wsel[:], wsel[:], wsel2[:])
                pgb = psp.tile([128, E], F32, tag="pmini")
                nc.tensor.matmul(pgb[:], ones_b[:], wsel[:],
                                 start=True, stop=True)
                wbc = actp.tile([128, E], F32, tag="wbc")
                nc.scalar.copy(wbc[:], pgb[:])

                macc = actp.tile([128, DC], F32, tag="macc")
                b2w = actp.tile([128, DC], F32, tag="b2w")
                be2 = biap.tile([128, E, DC], F32, tag="be2")
                nc.sync.dma_start(out=be2[:], in_=t_b_e2.ap()[l])
                for e in range(E):
                    w1 = we1p.tile([128, DC, CH, 128], BF, tag="we1")
                    nc.sync.dma_start(out=w1[:], in_=t_w_e1.ap()[l, e])
                    be1 = biap.tile([128, CH], F32, tag="be1")
                    nc.sync.dma_start(out=be1[:], in_=t_b_e1.ap()[l, e])
                    ph = psp.tile([128, CH], F32, tag="ph", bufs=1)
                    for j in range(CH):
                        for k in range(DC):
                            nc.tensor.matmul(ph[:, j:j + 1], w1[:, k, j, :],
                                             xn2[:, k:k + 1], start=(k == 0),
                                             stop=(k == DC - 1))
                    hsum = actp.tile([128, CH], F32, tag="hsum")
                    nc.vector.tensor_add(hsum[:], ph[:], be1[:])
                    hg = actp.tile([128, CH], BF, tag="hg")
                    nc.scalar.activation(hg[:], hsum[:], AF.Gelu)

                    w2 = we2p.tile([128, CH, DC, 128], BF, tag="we2")
                    nc.sync.dma_start(out=w2[:], in_=t_w_e2.ap()[l, e])
                    pe2 = psp.tile([128, DC], F32, tag="pe2", bufs=2)
                    for m in range(DC):
                        for k in range(CH):
                            nc.tensor.matmul(pe2[:, m:m + 1], w2[:, k, m, :],
                                             hg[:, k:k + 1], start=(k == 0),
                                             stop=(k == CH - 1))
                    if e == 0:
                        nc.vector.scalar_tensor_tensor(
                            out=macc[:], in0=pe2[:], scalar=wbc[:, 0:1],
                            in1=x_sb[:], op0=OP.mult, op1=OP.bypass)
                        nc.vector.scalar_tensor_tensor(
                            out=b2w[:], in0=be2[:, 0, :], scalar=wbc[:, 0:1],
                            in1=be2[:, 0, :], op0=OP.mult, op1=OP.bypass)
                    else:
                        nc.vector.scalar_tensor_tensor(
                            out=macc[:], in0=pe2[:], scalar=wbc[:, e:e + 1],
                            in1=macc[:], op0=OP.mult, op1=OP.add)
                        nc.vector.scalar_tensor_tensor(
                            out=b2w[:], in0=be2[:, e, :],
                            scalar=wbc[:, e:e + 1],
                            in1=b2w[:], op0=OP.mult, op1=OP.add)

                ar3 = ardp.tile([128, DC], F32, tag="ar3")
                nc.sync.dma_start(out=ar3[:], in_=macc[:])
                nc.gpsimd.collective_compute(
                    "AllReduce", OP.add, replica_groups=RG,
                    ins=[ar3[:]], outs=[ar3[:]])
                moe = actp.tile([128, DC], F32, tag="moe")
                nc.sync.dma_start(out=moe[:], in_=ar3[:])
                nc.vector.tensor_add(moe[:], moe[:], b2w[:])
                nc.vector.tensor_add(x_sb[:], x_sb[:], moe[:])

            # ---------- head ----------
            xf = rmsnorm(x_sb[:], "xf")
            whd = consts.tile([128, DC, NCLS], BF, tag="whd")
            nc.sync.dma_start(out=whd[:], in_=t_w_hd.ap())
            bhd = consts.tile([1, NCLS], F32, tag="bhd")
            nc.sync.dma_start(out=bhd[:], in_=t_b_hd.ap())
            phd = psp.tile([128, NCLS], F32, tag="pmini")
            for k in range(DC):
                nc.tensor.matmul(phd[0:1, :], xf[:, k:k + 1], whd[:, k, :],
                                 start=(k == 0), stop=(k == DC - 1))
            osb = actp.tile([1, NCLS], F32, tag="osb")
            nc.vector.tensor_add(osb[:], phd[0:1, :], bhd[:])
            nc.sync.dma_start(out=t_out.ap(), in_=osb[:])

    nc.compile()
    return nc


def get_nc(flags):
    if "nc" not in _CACHE:
        _CACHE["nc"] = _build()
    return _CACHE["nc"]


def kernel(**inputs):
    from concourse.bass_utils import run_bass_kernel_spmd
    sh, percore, flags = _prep(inputs)
    nc = get_nc(flags)
    in_maps = [{**sh, **pc} for pc in percore]
    res = run_bass_kernel_spmd(nc, in_maps, core_ids=list(range(NCORES)))
    row = np.asarray(res.results[0]["out"], np.float32).reshape(NCLS)
    return np.ascontiguousarray(
        np.broadcast_to(row[None, :], (B, NCLS))).astype(np.float32)
